# revision 31
# baseline (speedup 1.0000x reference)
"""BatchHardTriplet loss kernel for Trainium2 (8 NeuronCores, SPMD).

Strategy (v4 = v2 + band-min + tail fix)
----------------------------------------
The loss is row-permutation invariant, so the host packs WHOLE classes into 8
bins of exactly 1024 rows (greedy + swap repair) — every core's positives then
live in its own 1024 columns, and since rows are grouped by class, chunk mc's
positives live in a 256-col band [W_LO(mc), W_LO(mc)+256) of that window.
Rotating the candidate matrix per core puts the window at columns [0:1024).

Each core computes its [1024, 8192] sim block in 8 chunks of 128 rows into a
single [128, 4096] fp32 PSUM mega-tile (all 8 banks), two rounds of two
2048-col halves.  The -4*eq window mask is accumulated on the TensorEngine
(identity @ mask_fp8) over the full window.

PSUM exit bandwidth is the wall (only DVE + ScalarE have PSUM ports, both
1 elem/lane/cycle), so each sim exits exactly once:
 - band [W_LO:W_LO+256): DVE min reduce (hardest positive - 4)
 - window [0:1024]: DVE max reduce (masked positives never win)
 - everything else: split between DVE exact max reduces and ScalarE
   activation Exp(150*x - 75) with the built-in row accumulator — a
   log-sum-exp upper bound of the row max (bias ~3e-3 absolute), no SBUF
   round trip, no second engine.
ScalarE and DVE never read the same PSUM bank concurrently, and drains of
one half overlap fills of the other.

Host combines hn_sim = max(exact partials, 0.5 + ln(sum exp partials)/150),
applies validity and the final relu/mean.  Rel err vs reference ~6e-4.
"""

import sys
import numpy as np

sys.path.insert(0, "/opt/trn_rl_repo")

B = 8192
D = 128
M = 8            # cores
R = B // M       # 1024 rows per core
MC = R // 128    # 8 chunks of 128 rows per core
WINW = 1024      # window columns (the core's own rows)
BANDW = 256      # per-chunk positive band width
MARGIN = 0.3

T_LSE = 150.0    # log-sum-exp sharpness
C_LSE = 0.5      # centering: exp(T*(sim - C))

# drain split tuning (columns routed to each engine)
X2_ACT = 1024    # X2 half: ACT LSE on [0:X2_ACT], DVE direct on rest
X3_DVE = 1280    # X3 half: DVE direct on [0:X3_DVE], ACT LSE on rest
N_DUM = 14       # pre-ramp burst on iden; chunk0 fills extend the
                 # continuous-busy window so the ramp completes in-flight

_CACHE = {}


def _band_lo(mc):
    return min(max(mc * 128 - 64, 0), WINW - BANDW)


def _build_program():
    if "nc" in _CACHE:
        return _CACHE["nc"]

    import concourse.bacc as bacc
    import concourse.mybir as mybir
    from concourse import tile

    f32 = mybir.dt.float32
    bf16 = mybir.dt.bfloat16
    fp8 = mybir.dt.float8e4
    Exp = mybir.ActivationFunctionType.Exp
    AX = mybir.AxisListType.X
    amax = mybir.AluOpType.max
    amin = mybir.AluOpType.min

    nc = bacc.Bacc(None, target_bir_lowering=False)

    embA = nc.dram_tensor("embA", [D, B], bf16, kind="ExternalInput")
    masks = nc.dram_tensor("masks", [MC, 128, BANDW], fp8, kind="ExternalInput")
    iden = nc.dram_tensor("iden", [128, 128], fp8, kind="ExternalInput")
    outs = nc.dram_tensor("outs", [128, MC, 10], f32, kind="ExternalOutput")

    with tile.TileContext(nc) as tc:
        with (
            tc.tile_pool(name="big", bufs=1) as big,
            tc.tile_pool(name="sc", bufs=4) as sc,
            tc.tile_pool(name="st", bufs=1) as st,
            tc.tile_pool(name="ps", bufs=4, space="PSUM") as ps,
        ):
            # input DMAs in first-use order (transfers share HBM bandwidth)
            Id = big.tile([128, 128], fp8)
            nc.sync.dma_start(Id[:], iden[:])
            A = [big.tile([D, 1024], bf16, name=f"A{j}") for j in range(8)]
            Mk = [big.tile([128, BANDW], fp8, name=f"Mk{j}") for j in range(MC)]
            nc.sync.dma_start(A[0][:], embA[:, 0:1024])
            Bt = A[0]  # the core's own rows ARE rotated cols [0:1024)
            nc.sync.dma_start(Mk[0][:], masks[0])
            nc.sync.dma_start(A[1][:], embA[:, 1024:2048])
            nc.sync.dma_start(A[2][:], embA[:, 2048:3072])
            nc.sync.dma_start(A[3][:], embA[:, 3072:4096])
            nc.sync.dma_start(Mk[1][:], masks[1])
            nc.sync.dma_start(A[4][:], embA[:, 4096:5120])
            nc.sync.dma_start(A[5][:], embA[:, 5120:6144])
            nc.sync.dma_start(Mk[2][:], masks[2])
            nc.sync.dma_start(A[6][:], embA[:, 6144:7168])
            nc.sync.dma_start(A[7][:], embA[:, 7168:8192])
            for j in range(3, MC):
                nc.sync.dma_start(Mk[j][:], masks[j])

            out_t = st.tile([128, MC, 10], f32)
            bias_t = st.tile([128, 1], f32)
            nc.gpsimd.memset(bias_t[:], -T_LSE * C_LSE)
            nc.gpsimd.memset(out_t[:], 0.0)

            def lse(out_ap, in_ap, acc_ap):
                nc.scalar.activation(out_ap, in_ap, Exp,
                                     bias=bias_t[:], scale=T_LSE,
                                     accum_out=acc_ap)

            Gd = ps.tile([128, 1024], f32, tag="ps", name="Gd")
            for _ in range(N_DUM):
                nc.tensor.matmul(Gd[:, 0:128], Id[:], Id[:],
                                 start=True, stop=True, skip_group_check=True)

            # chunk0 is DMA-feed-bound (A tiles stream in at ~260GB/s);
            # interleave chunks 0 and 1 so chunk1's early groups (A0-A3,
            # already resident) overlap the wait for A4-A7
            sched = []
            for g in range(4):
                sched.append((0, g))
            for g in range(4):
                sched.append((1, g))
            for g in range(4, 8):
                sched.append((0, g))
            for g in range(4, 8):
                sched.append((1, g))
            for mc in range(2, MC):
                for g in range(8):
                    sched.append((mc, g))

            for mc, g in sched:
                lo = _band_lo(mc)
                lhsT = Bt[:, mc * 128:(mc + 1) * 128]
                if True:
                    G = ps.tile([128, 1024], f32, tag="ps", name=f"G{mc}_{g}")
                    for t in range(2):
                        col = g * 1024 + t * 512
                        a = A[col // 1024]
                        off = col % 1024
                        # band mask overlap with this bank (g0 only)
                        mlo = max(lo, col) if g == 0 else 0
                        mhi = min(lo + BANDW, col + 512) if g == 0 else 0
                        masked = mhi > mlo
                        nc.tensor.matmul(G[:, t * 512:(t + 1) * 512], lhsT,
                                         a[:, off:off + 512],
                                         start=True, stop=not masked)
                        if masked:
                            nc.tensor.matmul(G[:, mlo:mhi], Id[:],
                                             Mk[mc][:, mlo - lo:mhi - lo],
                                             start=False, stop=True)
                    if g == 0:
                        # band min (hardest pos - 4) + window max, both DVE
                        nc.vector.tensor_reduce(
                            out_t[:, mc, 0:1], G[:, lo:lo + BANDW],
                            axis=AX, op=amin)
                        nc.vector.tensor_reduce(
                            out_t[:, mc, 1:2], G[:], axis=AX, op=amax)
                    elif g == 6 and mc == MC - 1:
                        # swap engines for the last two drains: ACT takes g6
                        # so DVE can finish g7 right after the last fill
                        s = sc.tile([128, 1024], bf16, tag="sc",
                                    name=f"s{mc}_6")
                        lse(s[:], G[:], out_t[:, mc, 8:9])
                    elif g == 7 and mc == MC - 1:
                        nc.vector.tensor_reduce(
                            out_t[:, mc, 9:10], G[:], axis=AX, op=amax)
                    elif g % 2 == 1:  # odd groups: ScalarE LSE
                        s = sc.tile([128, 1024], bf16, tag="sc",
                                    name=f"s{mc}_{g}")
                        lse(s[:], G[:], out_t[:, mc, 5 + g // 2:6 + g // 2])
                    else:  # even groups 2,4,6: DVE direct max
                        nc.vector.tensor_reduce(
                            out_t[:, mc, g // 2:g // 2 + 1], G[:],
                            axis=AX, op=amax)

                if (mc, g) == (MC - 2, 7):
                    # drain chunks 0..6 results early; tail only ships mc=7
                    nc.sync.dma_start(outs[:, 0:MC - 1], out_t[:, 0:MC - 1])

            nc.sync.dma_start(outs[:, MC - 1:MC], out_t[:, MC - 1:MC])

    nc.compile()
    _CACHE["nc"] = nc
    return nc


def _pack_bins(labels, nbins=M, cap=R):
    """Assign whole classes to cores, each core exactly `cap` rows."""
    classes, counts = np.unique(labels, return_counts=True)
    order = np.argsort(-counts)
    bins = [[] for _ in range(nbins)]
    loads = [0] * nbins
    for idx in order:
        b = int(np.argmin(loads))
        bins[b].append(int(classes[idx]))
        loads[b] += int(counts[idx])
    size = {int(c): int(s) for c, s in zip(classes, counts)}
    for _ in range(100000):
        err = [l - cap for l in loads]
        if all(e == 0 for e in err):
            return bins
        over = max(range(nbins), key=lambda b: err[b])
        under = min(range(nbins), key=lambda b: err[b])
        cur = abs(err[over]) + abs(err[under])
        best = None
        for c1 in bins[over]:
            new = abs(err[over] - size[c1]) + abs(err[under] + size[c1])
            if new < cur and (best is None or new < best[0]):
                best = (new, c1, None)
        for c1 in bins[over]:
            for c2 in bins[under]:
                d = size[c1] - size[c2]
                if d <= 0:
                    continue
                new = abs(err[over] - d) + abs(err[under] + d)
                if new < cur and (best is None or new < best[0]):
                    best = (new, c1, c2)
        if best is None:
            return None
        _, c1, c2 = best
        bins[over].remove(c1)
        bins[under].append(c1)
        loads[over] -= size[c1]
        loads[under] += size[c1]
        if c2 is not None:
            bins[under].remove(c2)
            bins[over].append(c2)
            loads[under] -= size[c2]
            loads[over] += size[c2]
    return None


def _prep_inputs(emb, labels):
    """Class-pack rows into cores, rotate columns, build fp8 window masks."""
    import ml_dtypes

    emb = np.asarray(emb, dtype=np.float32)
    labels = np.asarray(labels)

    bins = _pack_bins(labels)
    assert bins is not None, "class bin packing failed"
    srt = np.argsort(labels, kind="stable")
    slab = labels[srt]
    bounds = np.searchsorted(slab, np.arange(int(labels.max()) + 2))
    cls_rows = {int(c): srt[bounds[c]:bounds[c + 1]] for c in np.unique(labels)}
    order = np.concatenate(
        [np.concatenate([cls_rows[c] for c in bins[b]]) for b in range(M)]
    )
    labs = labels[order]
    embs = emb[order]
    embT = np.ascontiguousarray(embs.T)  # [D, B]

    cnt_of = {int(c): int(s) for c, s in
              zip(*np.unique(labels, return_counts=True))}
    valid = np.array([cnt_of[int(l)] >= 2 for l in labs], dtype=bool)

    iden = np.eye(128, dtype=ml_dtypes.float8_e4m3)

    in_maps = []
    for c in range(M):
        r0 = c * R
        perm = (r0 + np.arange(B)) % B
        embA = np.ascontiguousarray(embT[:, perm]).astype(ml_dtypes.bfloat16)
        lab_win = labs[r0:r0 + R]
        lab_rows = lab_win.reshape(MC, 128)
        eq = lab_rows[:, :, None] == lab_win[None, None, :]
        mk = np.zeros((MC, 128, BANDW), dtype=np.float32)
        for mc in range(MC):
            lo = _band_lo(mc)
            # every positive must lie inside the per-chunk band
            assert int(eq[mc, :, lo:lo + BANDW].sum()) == int(eq[mc].sum()), \
                f"core {c} chunk {mc}: positives escape the band"
            mk[mc] = np.where(eq[mc, :, lo:lo + BANDW], -4.0, 0.0)
        in_maps.append({"embA": embA,
                        "masks": mk.astype(ml_dtypes.float8_e4m3),
                        "iden": iden})
    return in_maps, valid


def _postprocess(results, valid):
    minv = np.zeros(B, dtype=np.float32)
    maxv = np.zeros(B, dtype=np.float32)
    for c, res in enumerate(results):
        o = res["outs"]                        # [128, MC, 10]
        mn = o[:, :, 0]
        mx = np.maximum(o[:, :, 1:5].max(axis=2), o[:, :, 9])
        ls = o[:, :, 5:9].astype(np.float64).sum(axis=2)
        with np.errstate(divide="ignore"):
            lse = C_LSE + np.log(ls) / T_LSE
        hn_sim = np.maximum(mx, lse.astype(np.float32))
        for mc in range(MC):
            rows = slice(c * R + mc * 128, c * R + mc * 128 + 128)
            minv[rows] = mn[:, mc]
            maxv[rows] = hn_sim[:, mc]
    hp = 1.0 - (minv + 4.0)   # hardest positive distance
    hn = 1.0 - maxv           # hardest negative distance
    per_row = np.maximum(0.0, hp - hn + MARGIN)
    cnt = int(valid.sum())
    if cnt == 0:
        return np.float32(0.0)
    return np.float32(np.sum(per_row[valid]) / cnt)


def run_device(in_maps, trace=False):
    from concourse.bass_utils import run_bass_kernel_spmd

    nc = _build_program()
    return run_bass_kernel_spmd(nc, in_maps, list(range(M)), trace=trace)


def kernel(emb, labels):
    in_maps, valid = _prep_inputs(emb, labels)
    out = run_device(in_maps, trace=False)
    return _postprocess(out.results, valid)


if __name__ == "__main__":
    rng = np.random.default_rng(0)
    emb = rng.standard_normal((B, D)).astype(np.float32)
    emb /= np.linalg.norm(emb, axis=1, keepdims=True) + 1e-12
    labels = rng.integers(0, 512, B).astype(np.int32)
    print(kernel(emb, labels))
